# revision 37
# baseline (speedup 1.0000x reference)
"""Trainium2 Bass kernel: autoregressive graph generator (GNN encoder + LSTM + GNN decoder).

Sharding: 8-way tensor parallel over the LSTM hidden/gate dim. Each core holds
1/8 of the gate rows of W_hh (bf16, SBUF-resident) and computes its slice of the
gates; h is AllGathered (bf16) every step. The encoder SAGEConv (NF=10 -> H=2048)
composed with W_ih factors through a rank-20 bottleneck, so W_ih @ W_enc_{l,r} is
precomposed on the host and the whole x-side becomes a K=20 GEMM per step.
The mean aggregation is a fixed dense matrix A built from edge_index on the host.

All layouts on device are "T-layout": [feature/hidden dim (partitions), nodes (free)].
"""

import numpy as np
import ml_dtypes

import concourse.mybir as mybir
import concourse.tile as tile
from concourse import bacc, bass_utils
from concourse.bass import ts
from concourse.masks import make_identity

BF = ml_dtypes.bfloat16
F8 = ml_dtypes.float8_e4m3

N, NF, H, NG, K = 256, 10, 2048, 20, 10
NCORES = 8
HS = H // NCORES          # 256 hidden dims per core
GD = 4 * HS               # 1024 gate rows per core
MT = GD // 128            # 8 gate m-tiles per core
KT = H // 128             # 16 h k-tiles
NT = N // 128             # 2 node tiles
GEN = NG - K              # 10 generated steps

_PROG = [None]


def _emit_decoder_R(nc, pools, consts, t, vw_ps, hh_fill):
    """Gen-step decoder: R = [const4 | pad | xa | pad | p1] (96 rows, blocks
    at partition offsets 0/32/64), so the whole x-side of the gates is one
    k=96 GEMM (wstack @ R). hh_fill(i) emits chunks of the hh GEMM between
    the tail's PE ops so its DVE hops hide under them."""
    f32, bf16 = mybir.dt.float32, mybir.dt.bfloat16
    cpool, wpool, apool, gpool, spool = pools
    at, a2t, qr, r34, ident = (
        consts["at"], consts["a2t"], consts["qr"], consts["r34"],
        consts["ident"],
    )
    s = t - K
    vw_sb = wpool.tile([16, N], bf16, tag="vw", name=f"vwsb{t}")
    nc.vector.tensor_scalar_add(vw_sb[:], vw_ps[:], qr[:, s:s + 1])
    hh_fill(0)

    # transpose v'|w' -> non-T [N, 16] per node-tile
    vwT = []
    for j in range(NT):
        tp = spool.tile([128, 16], bf16, tag="sp", name=f"vwT{t}_{j}")
        nc.tensor.transpose(tp[:], vw_sb[:, ts(j, 128)], ident[:16, :16])
        tpsb = wpool.tile([128, 16], bf16, tag=f"vwTs{j}", name=f"vwTs{t}_{j}")
        nc.vector.tensor_copy(tpsb[:], tp[:])
        vwT.append(tpsb)
    hh_fill(1)

    # p1 = v@A2T + w@AT  [8, N]
    p1 = spool.tile([8, N], f32, tag="sp", name=f"p1_{t}")
    for kk in range(NT):
        nc.tensor.matmul(p1[:], vwT[kk][:, 0:8], a2t[kk][:],
                         start=(kk == 0), stop=False)
    for kk in range(NT):
        nc.tensor.matmul(p1[:], vwT[kk][:, 8:16], at[kk][:],
                         start=False, stop=(kk == NT - 1))
    # p2 = xa = v@AT + w; +w injected as identity matmuls (BIR only allows
    # partition offsets 0/32/64, so vw rows 8:16 cannot be read directly)
    p2 = spool.tile([8, N], f32, tag="sp", name=f"p2_{t}")
    for kk in range(NT):
        nc.tensor.matmul(p2[:], vwT[kk][:, 0:8], at[kk][:],
                         start=(kk == 0), stop=False)
    for kk in range(NT):
        nc.tensor.matmul(p2[:, ts(kk, 128)], vwT[kk][:, 8:16], ident[:],
                         start=False, stop=(kk == NT - 1))
    hh_fill(2)

    # R buffers are zeroed once at init; only the 3 blocks are rewritten.
    R = consts["Rbuf"][(t - K) % 2]
    nc.vector.tensor_copy(R[0:4, :], r34[:])
    nc.vector.tensor_copy(R[32:40, :], p2[:])
    nc.vector.tensor_copy(R[64:72, :], p1[:])
    return R


def _emit_xnext_out(nc, pools, consts, t, R):
    """Output assembly for gen step t: x_next = [st2 | xa.T] -> out_d[s]."""
    f32, bf16 = mybir.dt.float32, mybir.dt.bfloat16
    cpool, wpool, apool, gpool, spool = pools
    st2, ident, out_d = consts["st2"], consts["ident"], consts["out_d"]
    s = t - K
    for j in range(NT):
        xt = spool.tile([128, 8], bf16, tag="sp", name=f"xt{t}_{j}")
        nc.tensor.transpose(xt[:], R[32:40, ts(j, 128)],
                            ident[32:40, 32:40])
        xn = wpool.tile([128, NF], f32, tag=f"xn{j}", name=f"xn{t}_{j}")
        nc.vector.tensor_copy(xn[:, 0:2], st2[j][:])
        nc.vector.tensor_copy(xn[:, 2:NF], xt[:])
        nc.sync.dma_start(out_d[s, ts(j, 128), :], xn[:])


def _emit_step(nc, tc, pools, consts, t, h_tiles, c_prev):
    """One LSTM step: gate GEMMs + cell update + remote-DMA AllGather.

    h is exchanged via remote_dma_broadcast straight into every peer's SBUF
    hb[t%2] buffer (no DRAM staging, no ncfw collective, no reshape): step
    t's descriptors were pre-generated in step t-1's critical section, so
    the per-step cost is one trigger_dma + the SBUF->SBUF fabric transfer.
    Returns c_new.
    """
    f32, bf16 = mybir.dt.float32, mybir.dt.bfloat16
    fp8 = mybir.dt.float8e4
    cpool, wpool, apool, gpool, spool = pools
    whh, wc, bias, r20w = consts["whh"], consts["wc"], consts["bias"], consts["r20w"]
    wdec = consts["wdec"]
    Sig = mybir.ActivationFunctionType.Sigmoid
    Tanh = mybir.ActivationFunctionType.Tanh
    have_h = t > 0
    m_order = [0, 2, 4, 6, 1, 3, 5, 7]  # finish hidden-half 0 (i,f,g,o) early
    DR = mybir.MatmulPerfMode.DoubleRow

    hview = h_tiles[:].rearrange("p (a n) -> p a n", a=KT) if have_h else None

    def hhmm(g, j, m, start, stop=False):
        nc.tensor.matmul(
            g[:],
            whh[j][:].rearrange("p (s w) -> p s w", s=2)[:, :, ts(m, 128)],
            hview[:, 2 * j:2 * j + 2, :],
            start=start, stop=stop, perf_mode=DR)

    gp = {}
    if t < K:
        # Warm-up: two waves of 4 m-tiles, pair-outer inside each wave so
        # the first reshape quarter's pairs (cores 0-1) sweep all 4 m-tiles
        # before quarter 2 is needed (no reshape staircase stalls), while
        # only 4 PSUM groups are ever live per wave.
        for wave in ((4, 0, 2, 6), (5, 1, 3, 7)):
            for m in wave:
                g = gpool.tile([128, N], f32, tag="gp", name=f"gp{t}_{m}")
                nc.tensor.matmul(g[:], wc[:, ts(m, 128)],
                                 r20w[:, t * N:(t + 1) * N],
                                 start=True, stop=not have_h)
                gp[m] = g
            if have_h:
                for j in range(KT // 2):
                    for m in wave:
                        hhmm(gp[m], j, m, start=False,
                             stop=(j == KT // 2 - 1))
    elif t == NG - 1:
        # Final step: only the decoder output is consumed; the LSTM cell
        # update (gates + elementwise + h/c) would be dead code.
        vw_ps = spool.tile([16, N], f32, tag="sp", name=f"vwps{t}")
        for j in range(KT // 2):
            nc.tensor.matmul(vw_ps[:],
                             wdec[j][:].rearrange("p (s w) -> p s w", s=2),
                             hview[:, 2 * j:2 * j + 2, :],
                             start=(j == 0), stop=(j == KT // 2 - 1),
                             perf_mode=DR)
        R = _emit_decoder_R(nc, pools, consts, t, vw_ps, lambda i: None)
        _emit_xnext_out(nc, pools, consts, t, R)
        return None
    else:
        # Gen step: hh GEMMs first (pairs j<4 = cores 0-3, ready after the
        # first reshape half), decoder-R PE ops interleaved so their DVE
        # hops hide under hh work, x-side (k=96 wstack @ R) closes each
        # PSUM group last. <=4 gate groups live at once (pool bufs=5).
        def open_m(m):
            gp[m] = gpool.tile([128, N], f32, tag="gp", name=f"gp{t}_{m}")
            return gp[m]

        for m in (0, 2, 4, 6):
            open_m(m)
        for j in range(KT // 4):
            for m in (0, 2, 4, 6):
                hhmm(gp[m], j, m, start=(j == 0))
        vw_ps = spool.tile([16, N], f32, tag="sp", name=f"vwps{t}")
        for j in range(KT // 2):
            nc.tensor.matmul(vw_ps[:],
                             wdec[j][:].rearrange("p (s w) -> p s w", s=2),
                             hview[:, 2 * j:2 * j + 2, :],
                             start=(j == 0), stop=(j == KT // 2 - 1),
                             perf_mode=DR)

        fills = [(0, range(KT // 4, KT // 2)), (2, range(KT // 4, KT // 2)),
                 (4, range(KT // 4, KT // 2)), (6, range(KT // 4, KT // 2))]

        def hh_fill(i):
            m, js = fills[i]
            g = gp[m] if m in gp else open_m(m)
            for j in js:
                hhmm(g, j, m, start=(j == 0))

        R = _emit_decoder_R(nc, pools, consts, t, vw_ps, hh_fill)
        hh_fill(3)

        def close_m(m):
            nc.tensor.matmul(gp[m][:], consts["wstack"][:, ts(m, 128)], R[:],
                             start=False, stop=True)

        close_m(0)
        for mo, mc in ((1, 2), (3, 4), (5, 6)):
            g = open_m(mo)
            for j in range(KT // 2):
                hhmm(g, j, mo, start=(j == 0))
            close_m(mc)
        g = open_m(7)
        for j in range(KT // 2):
            hhmm(g, j, 7, start=(j == 0))
        for m in (5, 1, 3, 7):
            close_m(m)
        _emit_xnext_out(nc, pools, consts, t, R)

    def gsl(m):
        return gp[m][:]

    h2 = consts["h2buf"][t % 2]

    def bcol(m):
        return bias[:, (m * NG + t):(m * NG + t + 1)]

    # Phase 1: all 8 activations in m-close order (m_order closes the
    # half-0 gate tiles first, then half-1), so the scalar queue never
    # stalls on the cell-update chain of the other half.
    av = {}
    for hh in range(2):
        for idx, nm, fn in ((4, "tg", Tanh), (0, "si", Sig),
                            (2, "sf", Sig), (6, "so", Sig)):
            a = apool.tile([128, N], f32, tag=nm, name=f"{nm}{t}_{hh}")
            nc.scalar.activation(a[:], gsl(idx + hh), fn, bias=bcol(idx + hh))
            av[(hh, nm)] = a
    # Phase 2: cell updates (vector), both halves pipelined.
    c_new = []
    for hh in range(2):
        cn = wpool.tile([128, N], f32, tag=f"c{hh}", name=f"c{t}_{hh}")
        if t == 0:
            nc.vector.tensor_mul(cn[:], av[(hh, "si")][:], av[(hh, "tg")][:])
        else:
            p = apool.tile([128, N], f32, tag="p", name=f"p{t}_{hh}")
            nc.vector.tensor_mul(p[:], av[(hh, "si")][:], av[(hh, "tg")][:])
            tmp = apool.tile([128, N], f32, tag="tmp", name=f"tmp{t}_{hh}")
            nc.vector.tensor_mul(tmp[:], av[(hh, "sf")][:], c_prev[hh][:])
            nc.vector.tensor_add(cn[:], tmp[:], p[:])
        c_new.append(cn)
    # Phase 3: tanh(c) and h = sig(o)*tanh(c).
    tcs = []
    for hh in range(2):
        tc2 = apool.tile([128, N], f32, tag="tc", name=f"tc{t}_{hh}")
        nc.scalar.activation(tc2[:], c_new[hh][:], Tanh)
        tcs.append(tc2)
    for hh in range(2):
        nc.vector.tensor_mul(h2[:, ts(hh, N)], av[(hh, "so")][:], tcs[hh][:])

    if t < NG - 1:
        # Broadcast h2 into every core's hb[t%2] at column block (rank*512B).
        # The descriptors for this step were pre-generated (init section for
        # t=0, step t-1's critical section otherwise); here we only trigger.
        # signals_writable gives Tile the ordering edges: trigger waits for
        # the local h2 writes, and post-crit hands hb[t%2] to step t+1's
        # gate GEMMs only after all 8 sources' arrivals (rsem waits).
        rsem, lsem, psem = consts["rsem"], consts["lsem"], consts["psem"]
        hbb = consts["hbbuf"]
        with tc.tile_critical(no_gpsimd_drain=True):
            # wait_critical_data_deps defers the section's entry barrier to a
            # marker inside the executed arm: Pool enters early and runs the
            # Switch dispatch + descriptor generation concurrently with the
            # gate/cell compute; only the trigger waits for the h2 writes.
            for case in nc.gpsimd.Switch(consts["rank"], NCORES):
                nc.gpsimd.remote_dma_broadcast(
                    hbb[t % 2][:, ts(case, 2 * N)],
                    h2[:],
                    remote_sem=rsem[0],
                    local_sem=lsem,
                    rdests=[(0, k) for k in range(NCORES)],
                ).then_inc(psem, 1)
            nc.gpsimd.wait_ge(psem, t + 1)
            tc.wait_critical_data_deps()
            nc.gpsimd.trigger_dma(count=1)
            if t >= 1:
                nc.gpsimd.wait_ge(lsem, 16 * t)
            # all 8 sources bump the same sem (+2 each): one wait for the
            # full gather instead of eight per-source waits
            nc.gpsimd.wait_ge(rsem[0], 16 * (t + 1))
    return c_new


def _build_program():
    f32, bf16 = mybir.dt.float32, mybir.dt.bfloat16
    nc = bacc.Bacc("TRN2", target_bir_lowering=False, debug=False,
                   num_devices=NCORES)

    fp8 = mybir.dt.float8e4
    whhT_d = nc.dram_tensor("whhT", [H // 2, 2 * GD], fp8,
                            kind="ExternalInput").ap()
    wcT_d = nc.dram_tensor("wcT", [20, GD], bf16, kind="ExternalInput").ap()
    wstackT_d = nc.dram_tensor("wstackT", [96, GD], bf16,
                               kind="ExternalInput").ap()
    a2t_d = nc.dram_tensor("a2t", [N, N], bf16, kind="ExternalInput").ap()
    r34_d = nc.dram_tensor("r34", [4, N], bf16, kind="ExternalInput").ap()
    bias_d = nc.dram_tensor("biases", [128, MT * NG], f32, kind="ExternalInput").ap()
    at_d = nc.dram_tensor("at", [N, N], bf16, kind="ExternalInput").ap()
    wdec_d = nc.dram_tensor("wdecT", [H // 2, 32], fp8,
                            kind="ExternalInput").ap()
    qr_d = nc.dram_tensor("qr", [16, GEN], f32, kind="ExternalInput").ap()
    r20_d = nc.dram_tensor("rhs20w", [20, K * N], bf16, kind="ExternalInput").ap()
    st2_d = nc.dram_tensor("st2", [N, 2], f32, kind="ExternalInput").ap()
    out_d = nc.dram_tensor("gen", [GEN, N, NF], f32, kind="ExternalOutput").ap()

    rsem = [nc.alloc_semaphore(name=f"rsem{r}") for r in range(NCORES)]
    lsem = nc.alloc_semaphore(name="lsem")
    psem = nc.alloc_semaphore(name="psem")

    with tile.TileContext(nc) as tc:
        with (
            tc.tile_pool(name="const", bufs=1) as cpool,
            tc.tile_pool(name="work", bufs=2) as wpool,
            tc.tile_pool(name="act", bufs=3) as apool,
            tc.tile_pool(name="gp", bufs=5, space="PSUM") as gpool,
            tc.tile_pool(name="sp", bufs=3, space="PSUM") as spool,
        ):
            pools = (cpool, wpool, apool, gpool, spool)

            fp8 = mybir.dt.float8e4
            whh = []
            for k in range(KT // 2):
                w = cpool.tile([128, 2 * GD], fp8, tag=f"whh{k}", name=f"whh{k}")
                nc.sync.dma_start(w[:], whhT_d[ts(k, 128), :])
                whh.append(w)
            wc = cpool.tile([20, GD], bf16, tag="wc", name="wc")
            nc.sync.dma_start(wc[:], wcT_d[:])
            wstack = cpool.tile([96, GD], bf16, tag="wstack", name="wstack")
            nc.sync.dma_start(wstack[:], wstackT_d[:])
            a2t = []
            for k in range(NT):
                a = cpool.tile([128, N], bf16, tag=f"a2t{k}", name=f"a2t{k}")
                nc.sync.dma_start(a[:], a2t_d[ts(k, 128), :])
                a2t.append(a)
            r34 = cpool.tile([4, N], bf16, tag="r34", name="r34")
            nc.sync.dma_start(r34[:], r34_d[:])
            rbuf = []
            for i in range(2):
                r = cpool.tile([96, N], bf16, tag=f"Rb{i}", name=f"Rb{i}")
                nc.gpsimd.memset(r[:], 0.0)
                rbuf.append(r)
            at = []
            for k in range(NT):
                a = cpool.tile([128, N], bf16, tag=f"at{k}", name=f"at{k}")
                nc.sync.dma_start(a[:], at_d[ts(k, 128), :])
                at.append(a)
            wdec = []
            for k in range(KT // 2):
                w = cpool.tile([128, 32], fp8, tag=f"wdec{k}", name=f"wdec{k}")
                nc.sync.dma_start(w[:], wdec_d[ts(k, 128), :])
                wdec.append(w)
            bias = cpool.tile([128, MT * NG], f32, tag="bias", name="bias")
            nc.sync.dma_start(bias[:], bias_d[:])
            qr = cpool.tile([16, GEN], f32, tag="qr", name="qr")
            nc.sync.dma_start(qr[:], qr_d[:])
            r20w = cpool.tile([20, K * N], bf16, tag="r20w", name="r20w")
            nc.sync.dma_start(r20w[:], r20_d[:])
            st2 = []
            for j in range(NT):
                s = cpool.tile([128, 2], f32, tag=f"st2{j}", name=f"st2_{j}")
                nc.sync.dma_start(s[:], st2_d[ts(j, 128), :])
                st2.append(s)
            ident = cpool.tile([128, 128], bf16, tag="ident", name="ident")
            make_identity(nc, ident[:])

            # dedicated double-buffered send (h2) / receive (hb) tiles:
            # addresses must be compile-time stable, identical on all cores
            h2buf, hbbuf = [], []
            for i in range(2):
                h = cpool.tile([128, 2 * N], fp8, tag=f"h2b{i}", name=f"h2b{i}")
                h2buf.append(h)
                b = cpool.tile([128, KT * N], fp8, tag=f"hbb{i}", name=f"hbb{i}")
                nc.gpsimd.memset(b[:], 0.0)
                hbbuf.append(b)

            consts = dict(whh=whh, wc=wc, wstack=wstack, a2t=a2t, r34=r34,
                          Rbuf=rbuf, bias=bias, at=at, wdec=wdec,
                          qr=qr, r20w=r20w, st2=st2, ident=ident, out_d=out_d,
                          h2buf=h2buf, hbbuf=hbbuf, rsem=rsem, lsem=lsem,
                          psem=psem)

            # clear sems, then enter-kernel barrier (so no peer's remote
            # write can race a clear)
            with tc.tile_critical():
                for s in rsem + [lsem, psem]:
                    nc.gpsimd.sem_clear(s)
                nc.gpsimd.bir_kernel_barrier_wait(
                    replica_groups=[list(range(NCORES))])
                consts["rank"] = nc.gpsimd.partition_id()

            nc.vector.nop()
            c_prev = None
            for t in range(NG):
                h_tiles = hbbuf[(t - 1) % 2] if t > 0 else None
                c_prev = _emit_step(nc, tc, pools, consts, t, h_tiles, c_prev)
    nc.compile()
    return nc


def _host_tensors(inputs):
    """All host-side preprocessing: A matrix, weight composition, per-core shards."""
    f32 = np.float32
    kg = np.asarray(inputs["known_graphs"], f32)
    ei = np.asarray(inputs["edge_index"])
    W_enc_l = np.asarray(inputs["W_enc_l"], f32)
    b_enc_l = np.asarray(inputs["b_enc_l"], f32)
    W_enc_r = np.asarray(inputs["W_enc_r"], f32)
    pos = np.asarray(inputs["pos_emb"], f32)
    W_ih = np.asarray(inputs["W_ih"], f32)
    W_hh = np.asarray(inputs["W_hh"], f32)
    b_ih = np.asarray(inputs["b_ih"], f32)
    b_hh = np.asarray(inputs["b_hh"], f32)
    W_dec_l = np.asarray(inputs["W_dec_l"], f32)
    b_dec_l = np.asarray(inputs["b_dec_l"], f32)
    W_dec_r = np.asarray(inputs["W_dec_r"], f32)

    src, dst = np.asarray(ei[0]), np.asarray(ei[1])
    C = np.zeros((N, N), np.float64)
    np.add.at(C, (dst, src), 1.0)
    cnt = C.sum(1)
    A = (C / np.maximum(cnt, 1.0)[:, None]).astype(f32)

    c64 = np.float64
    Wc1 = W_ih.astype(c64) @ W_enc_l.astype(c64)          # [4H, NF]
    Wc2 = W_ih.astype(c64) @ W_enc_r.astype(c64)
    Wc = np.concatenate([Wc1, Wc2], 1)                    # [4H, 20]
    # bias_t = W_ih @ (b_enc_l + pe_t) + b_ih + b_hh  -> [NG, 4H]
    bias_all = (W_ih.astype(c64) @ (b_enc_l.astype(c64)[:, None] + pos.astype(c64).T)).T \
        + b_ih.astype(c64) + b_hh.astype(c64)
    bias_all = bias_all.astype(f32)
    A2 = (A.astype(c64) @ A.astype(c64))
    # decoder pe folds: [16, GEN]
    qr = np.concatenate([
        (pos[K:NG].astype(c64) @ W_dec_l.T.astype(c64)).T,
        (pos[K:NG].astype(c64) @ W_dec_r.T.astype(c64)).T
        + b_dec_l.astype(c64)[:, None],
    ], 0).astype(f32)

    # warm-up rhs20: [20, K*N], col index t*N + i
    mean_w = np.einsum("ij,tjf->tif", A.astype(c64), kg.astype(c64))  # [K, N, NF]
    r20w = np.concatenate([
        np.transpose(mean_w, (2, 0, 1)).reshape(NF, -1),
        np.transpose(kg.astype(c64), (2, 0, 1)).reshape(NF, -1),
    ], 0).astype(f32)

    # DoubleRow pair packing: [KT/2 * 128, 2*cols], row j*128+p holds
    # global k-tiles (2j, 2j+1) side by side along the free dim
    def pack_pairs(wT):  # wT [H, cols] -> [H/2, 2*cols]
        cols = wT.shape[1]
        return np.ascontiguousarray(
            wT.reshape(KT // 2, 2, 128, cols).transpose(0, 2, 1, 3)
            .reshape(H // 2, 2 * cols))

    wdecT = np.concatenate([W_dec_l, W_dec_r], 0).T        # [H, 16]
    st2v = kg[-1, :, :2].astype(c64)                       # [N, 2]
    r34 = np.concatenate([(A.astype(c64) @ st2v).T, st2v.T], 0)  # [4, N]
    shared = {
        "at": np.ascontiguousarray(A.T).astype(BF),
        "a2t": np.ascontiguousarray(A2.T).astype(np.float32).astype(BF),
        "r34": np.ascontiguousarray(r34).astype(np.float32).astype(BF),
        "wdecT": pack_pairs(wdecT).astype(F8),
        "qr": np.ascontiguousarray(qr),
        "rhs20w": np.ascontiguousarray(r20w).astype(BF),
        "st2": np.ascontiguousarray(kg[-1, :, :2]),
    }

    in_maps = []
    for c in range(NCORES):
        idx = np.concatenate([np.arange(g * H + c * HS, g * H + (c + 1) * HS)
                              for g in range(4)])
        whhT = pack_pairs(W_hh[idx, :].T).astype(F8)                  # [H/2, 2GD]
        wcT = np.ascontiguousarray(Wc[idx, :].T).astype(BF)           # [20, GD]
        wst = np.zeros((4 * H, 96), np.float32)
        wst[:, 0:2] = Wc[:, 0:2]
        wst[:, 2:4] = Wc[:, NF:NF + 2]
        wst[:, 32:40] = Wc[:, NF + 2:NF + NF]
        wst[:, 64:72] = Wc[:, 2:NF]
        wstackT = np.ascontiguousarray(wst[idx, :].T).astype(BF)
        bc = bias_all[:, idx].T                                       # [GD, NG]
        bt = np.ascontiguousarray(
            bc.reshape(MT, 128, NG).transpose(1, 0, 2).reshape(128, MT * NG))
        in_maps.append({
            "whhT": whhT, "wcT": wcT, "wstackT": wstackT, "biases": bt,
            **shared,
        })
    return in_maps


def kernel(**inputs):
    if _PROG[0] is None:
        _PROG[0] = _build_program()
    nc = _PROG[0]
    in_maps = _host_tensors(inputs)
    res = bass_utils.run_bass_kernel_spmd(
        nc, in_maps, core_ids=list(range(NCORES)))
    return np.ascontiguousarray(res.results[0]["gen"]).astype(np.float32)


# exposed for test.py profiling
def run_profiled(inputs, **kwargs):
    if _PROG[0] is None:
        _PROG[0] = _build_program()
    in_maps = _host_tensors(inputs)
    return bass_utils.run_bass_kernel_spmd(
        _PROG[0], in_maps, core_ids=list(range(NCORES)), **kwargs)



# revision 39
# speedup vs baseline: 1.0217x; 1.0217x over previous
"""Trainium2 Bass kernel: autoregressive graph generator (GNN encoder + LSTM + GNN decoder).

Sharding: 8-way tensor parallel over the LSTM hidden/gate dim. Each core holds
1/8 of the gate rows of W_hh (bf16, SBUF-resident) and computes its slice of the
gates; h is AllGathered (bf16) every step. The encoder SAGEConv (NF=10 -> H=2048)
composed with W_ih factors through a rank-20 bottleneck, so W_ih @ W_enc_{l,r} is
precomposed on the host and the whole x-side becomes a K=20 GEMM per step.
The mean aggregation is a fixed dense matrix A built from edge_index on the host.

All layouts on device are "T-layout": [feature/hidden dim (partitions), nodes (free)].
"""

import numpy as np
import ml_dtypes

import concourse.mybir as mybir
import concourse.tile as tile
from concourse import bacc, bass_utils
from concourse.bass import ts
from concourse.masks import make_identity

BF = ml_dtypes.bfloat16
F8 = ml_dtypes.float8_e4m3

N, NF, H, NG, K = 256, 10, 2048, 20, 10
NCORES = 8
HS = H // NCORES          # 256 hidden dims per core
GD = 4 * HS               # 1024 gate rows per core
MT = GD // 128            # 8 gate m-tiles per core
KT = H // 128             # 16 h k-tiles
NT = N // 128             # 2 node tiles
GEN = NG - K              # 10 generated steps

_PROG = [None]


def _emit_decoder_R(nc, pools, consts, t, vw_ps, hh_fill):
    """Gen-step decoder: R = [const4 | pad | xa | pad | p1] (96 rows, blocks
    at partition offsets 0/32/64), so the whole x-side of the gates is one
    k=96 GEMM (wstack @ R). hh_fill(i) emits chunks of the hh GEMM between
    the tail's PE ops so its DVE hops hide under them."""
    f32, bf16 = mybir.dt.float32, mybir.dt.bfloat16
    cpool, wpool, apool, gpool, spool = pools
    at, a2t, qr, r34, ident = (
        consts["at"], consts["a2t"], consts["qr"], consts["r34"],
        consts["ident"],
    )
    s = t - K
    vw_sb = wpool.tile([16, N], bf16, tag="vw", name=f"vwsb{t}")
    nc.vector.tensor_scalar_add(vw_sb[:], vw_ps[:], qr[:, s:s + 1])
    hh_fill(0)

    # transpose v'|w' -> non-T [N, 16] per node-tile
    vwT = []
    for j in range(NT):
        tp = spool.tile([128, 16], bf16, tag="sp", name=f"vwT{t}_{j}")
        nc.tensor.transpose(tp[:], vw_sb[:, ts(j, 128)], ident[:16, :16])
        tpsb = wpool.tile([128, 16], bf16, tag=f"vwTs{j}", name=f"vwTs{t}_{j}")
        nc.vector.tensor_copy(tpsb[:], tp[:])
        vwT.append(tpsb)
    hh_fill(1)

    # p1 = v@A2T + w@AT  [8, N]
    p1 = spool.tile([8, N], f32, tag="sp", name=f"p1_{t}")
    for kk in range(NT):
        nc.tensor.matmul(p1[:], vwT[kk][:, 0:8], a2t[kk][:],
                         start=(kk == 0), stop=False)
    for kk in range(NT):
        nc.tensor.matmul(p1[:], vwT[kk][:, 8:16], at[kk][:],
                         start=False, stop=(kk == NT - 1))
    # p2 = xa = v@AT + w; +w injected as identity matmuls (BIR only allows
    # partition offsets 0/32/64, so vw rows 8:16 cannot be read directly)
    p2 = spool.tile([8, N], f32, tag="sp", name=f"p2_{t}")
    for kk in range(NT):
        nc.tensor.matmul(p2[:], vwT[kk][:, 0:8], at[kk][:],
                         start=(kk == 0), stop=False)
    for kk in range(NT):
        nc.tensor.matmul(p2[:, ts(kk, 128)], vwT[kk][:, 8:16], ident[:],
                         start=False, stop=(kk == NT - 1))
    hh_fill(2)

    # R buffers are zeroed once at init; only the 3 blocks are rewritten.
    R = consts["Rbuf"][(t - K) % 2]
    nc.vector.tensor_copy(R[0:4, :], r34[:])
    nc.vector.tensor_copy(R[32:40, :], p2[:])
    nc.vector.tensor_copy(R[64:72, :], p1[:])
    return R


def _emit_xnext_out(nc, pools, consts, t, R):
    """Output assembly for gen step t: x_next = [st2 | xa.T] -> out_d[s]."""
    f32, bf16 = mybir.dt.float32, mybir.dt.bfloat16
    cpool, wpool, apool, gpool, spool = pools
    st2, ident, out_d = consts["st2"], consts["ident"], consts["out_d"]
    s = t - K
    for j in range(NT):
        xt = spool.tile([128, 8], bf16, tag="sp", name=f"xt{t}_{j}")
        nc.tensor.transpose(xt[:], R[32:40, ts(j, 128)],
                            ident[32:40, 32:40])
        xn = wpool.tile([128, NF], f32, tag=f"xn{j}", name=f"xn{t}_{j}")
        nc.vector.tensor_copy(xn[:, 0:2], st2[j][:])
        nc.vector.tensor_copy(xn[:, 2:NF], xt[:])
        nc.sync.dma_start(out_d[s, ts(j, 128), :], xn[:])


def _emit_step(nc, tc, pools, consts, t, h_tiles, c_prev):
    """One LSTM step: gate GEMMs + cell update + remote-DMA AllGather.

    h is exchanged via remote_dma_broadcast straight into every peer's SBUF
    hb[t%2] buffer (no DRAM staging, no ncfw collective, no reshape): step
    t's descriptors were pre-generated in step t-1's critical section, so
    the per-step cost is one trigger_dma + the SBUF->SBUF fabric transfer.
    Returns c_new.
    """
    f32, bf16 = mybir.dt.float32, mybir.dt.bfloat16
    fp8 = mybir.dt.float8e4
    cpool, wpool, apool, gpool, spool = pools
    whh, wc, bias, r20w = consts["whh"], consts["wc"], consts["bias"], consts["r20w"]
    wdec = consts["wdec"]
    Sig = mybir.ActivationFunctionType.Sigmoid
    Tanh = mybir.ActivationFunctionType.Tanh
    have_h = t > 0
    m_order = [0, 2, 4, 6, 1, 3, 5, 7]  # finish hidden-half 0 (i,f,g,o) early
    DR = mybir.MatmulPerfMode.DoubleRow

    hview = h_tiles[:].rearrange("p (a n) -> p a n", a=KT) if have_h else None

    def hhmm(g, j, m, start, stop=False):
        nc.tensor.matmul(
            g[:],
            whh[j][:].rearrange("p (s w) -> p s w", s=2)[:, :, ts(m, 128)],
            hview[:, 2 * j:2 * j + 2, :],
            start=start, stop=stop, perf_mode=DR)

    gp = {}
    if t < K:
        # Warm-up: two waves of 4 m-tiles, pair-outer inside each wave so
        # the first reshape quarter's pairs (cores 0-1) sweep all 4 m-tiles
        # before quarter 2 is needed (no reshape staircase stalls), while
        # only 4 PSUM groups are ever live per wave.
        for wave in ((0, 2, 4, 6), (1, 3, 5, 7)):
            for m in wave:
                g = gpool.tile([128, N], f32, tag="gp", name=f"gp{t}_{m}")
                nc.tensor.matmul(g[:], wc[:, ts(m, 128)],
                                 r20w[:, t * N:(t + 1) * N],
                                 start=True, stop=not have_h)
                gp[m] = g
            if have_h:
                for j in range(KT // 2):
                    for m in wave:
                        hhmm(gp[m], j, m, start=False,
                             stop=(j == KT // 2 - 1))
    elif t == NG - 1:
        # Final step: only the decoder output is consumed; the LSTM cell
        # update (gates + elementwise + h/c) would be dead code.
        vw_ps = spool.tile([16, N], f32, tag="sp", name=f"vwps{t}")
        for j in range(KT // 2):
            nc.tensor.matmul(vw_ps[:],
                             wdec[j][:].rearrange("p (s w) -> p s w", s=2),
                             hview[:, 2 * j:2 * j + 2, :],
                             start=(j == 0), stop=(j == KT // 2 - 1),
                             perf_mode=DR)
        R = _emit_decoder_R(nc, pools, consts, t, vw_ps, lambda i: None)
        _emit_xnext_out(nc, pools, consts, t, R)
        return None
    else:
        # Gen step: hh GEMMs first (pairs j<4 = cores 0-3, ready after the
        # first reshape half), decoder-R PE ops interleaved so their DVE
        # hops hide under hh work, x-side (k=96 wstack @ R) closes each
        # PSUM group last. <=4 gate groups live at once (pool bufs=5).
        def open_m(m):
            gp[m] = gpool.tile([128, N], f32, tag="gp", name=f"gp{t}_{m}")
            return gp[m]

        for m in (0, 2, 4, 6):
            open_m(m)
        for j in range(KT // 4):
            for m in (0, 2, 4, 6):
                hhmm(gp[m], j, m, start=(j == 0))
        vw_ps = spool.tile([16, N], f32, tag="sp", name=f"vwps{t}")
        for j in range(KT // 2):
            nc.tensor.matmul(vw_ps[:],
                             wdec[j][:].rearrange("p (s w) -> p s w", s=2),
                             hview[:, 2 * j:2 * j + 2, :],
                             start=(j == 0), stop=(j == KT // 2 - 1),
                             perf_mode=DR)

        fills = [(0, range(KT // 4, KT // 2)), (2, range(KT // 4, KT // 2)),
                 (4, range(KT // 4, KT // 2)), (6, range(KT // 4, KT // 2))]

        def hh_fill(i):
            m, js = fills[i]
            g = gp[m] if m in gp else open_m(m)
            for j in js:
                hhmm(g, j, m, start=(j == 0))

        R = _emit_decoder_R(nc, pools, consts, t, vw_ps, hh_fill)
        hh_fill(3)

        def close_m(m):
            nc.tensor.matmul(gp[m][:], consts["wstack"][:, ts(m, 128)], R[:],
                             start=False, stop=True)

        close_m(0)
        for mo, mc in ((1, 2), (3, 4), (5, 6)):
            g = open_m(mo)
            for j in range(KT // 2):
                hhmm(g, j, mo, start=(j == 0))
            close_m(mc)
        g = open_m(7)
        for j in range(KT // 2):
            hhmm(g, j, 7, start=(j == 0))
        for m in (1, 3, 5, 7):
            close_m(m)
        _emit_xnext_out(nc, pools, consts, t, R)

    def gsl(m):
        return gp[m][:]

    h2 = consts["h2buf"][t % 2]

    def bcol(m):
        return bias[:, (m * NG + t):(m * NG + t + 1)]

    # Phase 1: all 8 activations in m-close order (m_order closes the
    # half-0 gate tiles first, then half-1), so the scalar queue never
    # stalls on the cell-update chain of the other half.
    av = {}
    for hh in range(2):
        for idx, nm, fn in ((0, "si", Sig), (2, "sf", Sig),
                            (4, "tg", Tanh), (6, "so", Sig)):
            a = apool.tile([128, N], f32, tag=nm, name=f"{nm}{t}_{hh}")
            nc.scalar.activation(a[:], gsl(idx + hh), fn, bias=bcol(idx + hh))
            av[(hh, nm)] = a
    # Phase 2: cell updates (vector), both halves pipelined.
    c_new = []
    for hh in range(2):
        cn = wpool.tile([128, N], f32, tag=f"c{hh}", name=f"c{t}_{hh}")
        if t == 0:
            nc.vector.tensor_mul(cn[:], av[(hh, "si")][:], av[(hh, "tg")][:])
        else:
            p = apool.tile([128, N], f32, tag="p", name=f"p{t}_{hh}")
            nc.vector.tensor_mul(p[:], av[(hh, "si")][:], av[(hh, "tg")][:])
            tmp = apool.tile([128, N], f32, tag="tmp", name=f"tmp{t}_{hh}")
            nc.vector.tensor_mul(tmp[:], av[(hh, "sf")][:], c_prev[hh][:])
            nc.vector.tensor_add(cn[:], tmp[:], p[:])
        c_new.append(cn)
    # Phase 3: tanh(c) and h = sig(o)*tanh(c).
    tcs = []
    for hh in range(2):
        tc2 = apool.tile([128, N], f32, tag="tc", name=f"tc{t}_{hh}")
        nc.scalar.activation(tc2[:], c_new[hh][:], Tanh)
        tcs.append(tc2)
    for hh in range(2):
        nc.vector.tensor_mul(h2[:, ts(hh, N)], av[(hh, "so")][:], tcs[hh][:])

    if t < NG - 1:
        # Broadcast h2 into every core's hb[t%2] at column block (rank*512B).
        # The descriptors for this step were pre-generated (init section for
        # t=0, step t-1's critical section otherwise); here we only trigger.
        # signals_writable gives Tile the ordering edges: trigger waits for
        # the local h2 writes, and post-crit hands hb[t%2] to step t+1's
        # gate GEMMs only after all 8 sources' arrivals (rsem waits).
        rsem, lsem, psem = consts["rsem"], consts["lsem"], consts["psem"]
        hbb = consts["hbbuf"]
        with tc.tile_critical(no_gpsimd_drain=True):
            # wait_critical_data_deps defers the section's entry barrier to a
            # marker inside the executed arm: Pool enters early and runs the
            # Switch dispatch + descriptor generation concurrently with the
            # gate/cell compute; only the trigger waits for the h2 writes.
            for case in nc.gpsimd.Switch(consts["rank"], NCORES):
                nc.gpsimd.remote_dma_broadcast(
                    hbb[t % 2][:, ts(case, 2 * N)],
                    h2[:],
                    remote_sem=rsem[0],
                    local_sem=lsem,
                    rdests=[(0, k) for k in range(NCORES)],
                ).then_inc(psem, 1)
            nc.gpsimd.wait_ge(psem, t + 1)
            # lsem guard (send t-2 drained) retires during compute, off the
            # post-trigger critical path
            if t >= 1:
                nc.gpsimd.wait_ge(lsem, 16 * t)
            tc.wait_critical_data_deps()
            nc.gpsimd.trigger_dma(count=1)
            # all 8 sources bump the same sem (+2 each): one wait for the
            # full gather instead of eight per-source waits
            nc.gpsimd.wait_ge(rsem[0], 16 * (t + 1))
    return c_new


def _build_program():
    f32, bf16 = mybir.dt.float32, mybir.dt.bfloat16
    nc = bacc.Bacc("TRN2", target_bir_lowering=False, debug=False,
                   num_devices=NCORES)

    fp8 = mybir.dt.float8e4
    whhT_d = nc.dram_tensor("whhT", [H // 2, 2 * GD], fp8,
                            kind="ExternalInput").ap()
    wcT_d = nc.dram_tensor("wcT", [20, GD], bf16, kind="ExternalInput").ap()
    wstackT_d = nc.dram_tensor("wstackT", [96, GD], bf16,
                               kind="ExternalInput").ap()
    a2t_d = nc.dram_tensor("a2t", [N, N], bf16, kind="ExternalInput").ap()
    r34_d = nc.dram_tensor("r34", [4, N], bf16, kind="ExternalInput").ap()
    bias_d = nc.dram_tensor("biases", [128, MT * NG], f32, kind="ExternalInput").ap()
    at_d = nc.dram_tensor("at", [N, N], bf16, kind="ExternalInput").ap()
    wdec_d = nc.dram_tensor("wdecT", [H // 2, 32], fp8,
                            kind="ExternalInput").ap()
    qr_d = nc.dram_tensor("qr", [16, GEN], f32, kind="ExternalInput").ap()
    r20_d = nc.dram_tensor("rhs20w", [20, K * N], bf16, kind="ExternalInput").ap()
    st2_d = nc.dram_tensor("st2", [N, 2], f32, kind="ExternalInput").ap()
    out_d = nc.dram_tensor("gen", [GEN, N, NF], f32, kind="ExternalOutput").ap()

    rsem = [nc.alloc_semaphore(name=f"rsem{r}") for r in range(NCORES)]
    lsem = nc.alloc_semaphore(name="lsem")
    psem = nc.alloc_semaphore(name="psem")

    with tile.TileContext(nc) as tc:
        with (
            tc.tile_pool(name="const", bufs=1) as cpool,
            tc.tile_pool(name="work", bufs=2) as wpool,
            tc.tile_pool(name="act", bufs=3) as apool,
            tc.tile_pool(name="gp", bufs=5, space="PSUM") as gpool,
            tc.tile_pool(name="sp", bufs=3, space="PSUM") as spool,
        ):
            pools = (cpool, wpool, apool, gpool, spool)

            fp8 = mybir.dt.float8e4
            whh = []
            for k in range(KT // 2):
                w = cpool.tile([128, 2 * GD], fp8, tag=f"whh{k}", name=f"whh{k}")
                nc.sync.dma_start(w[:], whhT_d[ts(k, 128), :])
                whh.append(w)
            wc = cpool.tile([20, GD], bf16, tag="wc", name="wc")
            nc.sync.dma_start(wc[:], wcT_d[:])
            wstack = cpool.tile([96, GD], bf16, tag="wstack", name="wstack")
            nc.sync.dma_start(wstack[:], wstackT_d[:])
            a2t = []
            for k in range(NT):
                a = cpool.tile([128, N], bf16, tag=f"a2t{k}", name=f"a2t{k}")
                nc.sync.dma_start(a[:], a2t_d[ts(k, 128), :])
                a2t.append(a)
            r34 = cpool.tile([4, N], bf16, tag="r34", name="r34")
            nc.sync.dma_start(r34[:], r34_d[:])
            rbuf = []
            for i in range(2):
                r = cpool.tile([96, N], bf16, tag=f"Rb{i}", name=f"Rb{i}")
                nc.gpsimd.memset(r[:], 0.0)
                rbuf.append(r)
            at = []
            for k in range(NT):
                a = cpool.tile([128, N], bf16, tag=f"at{k}", name=f"at{k}")
                nc.sync.dma_start(a[:], at_d[ts(k, 128), :])
                at.append(a)
            wdec = []
            for k in range(KT // 2):
                w = cpool.tile([128, 32], fp8, tag=f"wdec{k}", name=f"wdec{k}")
                nc.sync.dma_start(w[:], wdec_d[ts(k, 128), :])
                wdec.append(w)
            bias = cpool.tile([128, MT * NG], f32, tag="bias", name="bias")
            nc.sync.dma_start(bias[:], bias_d[:])
            qr = cpool.tile([16, GEN], f32, tag="qr", name="qr")
            nc.sync.dma_start(qr[:], qr_d[:])
            r20w = cpool.tile([20, K * N], bf16, tag="r20w", name="r20w")
            nc.sync.dma_start(r20w[:], r20_d[:])
            st2 = []
            for j in range(NT):
                s = cpool.tile([128, 2], f32, tag=f"st2{j}", name=f"st2_{j}")
                nc.sync.dma_start(s[:], st2_d[ts(j, 128), :])
                st2.append(s)
            ident = cpool.tile([128, 128], bf16, tag="ident", name="ident")
            make_identity(nc, ident[:])

            # dedicated double-buffered send (h2) / receive (hb) tiles:
            # addresses must be compile-time stable, identical on all cores
            h2buf, hbbuf = [], []
            for i in range(2):
                h = cpool.tile([128, 2 * N], fp8, tag=f"h2b{i}", name=f"h2b{i}")
                h2buf.append(h)
                b = cpool.tile([128, KT * N], fp8, tag=f"hbb{i}", name=f"hbb{i}")
                nc.gpsimd.memset(b[:], 0.0)
                hbbuf.append(b)

            consts = dict(whh=whh, wc=wc, wstack=wstack, a2t=a2t, r34=r34,
                          Rbuf=rbuf, bias=bias, at=at, wdec=wdec,
                          qr=qr, r20w=r20w, st2=st2, ident=ident, out_d=out_d,
                          h2buf=h2buf, hbbuf=hbbuf, rsem=rsem, lsem=lsem,
                          psem=psem)

            # clear sems, then enter-kernel barrier (so no peer's remote
            # write can race a clear)
            with tc.tile_critical():
                for s in rsem + [lsem, psem]:
                    nc.gpsimd.sem_clear(s)
                nc.gpsimd.bir_kernel_barrier_wait(
                    replica_groups=[list(range(NCORES))])
                consts["rank"] = nc.gpsimd.partition_id()

            nc.vector.nop()
            c_prev = None
            for t in range(NG):
                h_tiles = hbbuf[(t - 1) % 2] if t > 0 else None
                c_prev = _emit_step(nc, tc, pools, consts, t, h_tiles, c_prev)
    nc.compile()
    return nc


def _host_tensors(inputs):
    """All host-side preprocessing: A matrix, weight composition, per-core shards."""
    f32 = np.float32
    kg = np.asarray(inputs["known_graphs"], f32)
    ei = np.asarray(inputs["edge_index"])
    W_enc_l = np.asarray(inputs["W_enc_l"], f32)
    b_enc_l = np.asarray(inputs["b_enc_l"], f32)
    W_enc_r = np.asarray(inputs["W_enc_r"], f32)
    pos = np.asarray(inputs["pos_emb"], f32)
    W_ih = np.asarray(inputs["W_ih"], f32)
    W_hh = np.asarray(inputs["W_hh"], f32)
    b_ih = np.asarray(inputs["b_ih"], f32)
    b_hh = np.asarray(inputs["b_hh"], f32)
    W_dec_l = np.asarray(inputs["W_dec_l"], f32)
    b_dec_l = np.asarray(inputs["b_dec_l"], f32)
    W_dec_r = np.asarray(inputs["W_dec_r"], f32)

    src, dst = np.asarray(ei[0]), np.asarray(ei[1])
    C = np.zeros((N, N), np.float64)
    np.add.at(C, (dst, src), 1.0)
    cnt = C.sum(1)
    A = (C / np.maximum(cnt, 1.0)[:, None]).astype(f32)

    c64 = np.float64
    Wc1 = W_ih.astype(c64) @ W_enc_l.astype(c64)          # [4H, NF]
    Wc2 = W_ih.astype(c64) @ W_enc_r.astype(c64)
    Wc = np.concatenate([Wc1, Wc2], 1)                    # [4H, 20]
    # bias_t = W_ih @ (b_enc_l + pe_t) + b_ih + b_hh  -> [NG, 4H]
    bias_all = (W_ih.astype(c64) @ (b_enc_l.astype(c64)[:, None] + pos.astype(c64).T)).T \
        + b_ih.astype(c64) + b_hh.astype(c64)
    bias_all = bias_all.astype(f32)
    A2 = (A.astype(c64) @ A.astype(c64))
    # decoder pe folds: [16, GEN]
    qr = np.concatenate([
        (pos[K:NG].astype(c64) @ W_dec_l.T.astype(c64)).T,
        (pos[K:NG].astype(c64) @ W_dec_r.T.astype(c64)).T
        + b_dec_l.astype(c64)[:, None],
    ], 0).astype(f32)

    # warm-up rhs20: [20, K*N], col index t*N + i
    mean_w = np.einsum("ij,tjf->tif", A.astype(c64), kg.astype(c64))  # [K, N, NF]
    r20w = np.concatenate([
        np.transpose(mean_w, (2, 0, 1)).reshape(NF, -1),
        np.transpose(kg.astype(c64), (2, 0, 1)).reshape(NF, -1),
    ], 0).astype(f32)

    # DoubleRow pair packing: [KT/2 * 128, 2*cols], row j*128+p holds
    # global k-tiles (2j, 2j+1) side by side along the free dim
    def pack_pairs(wT):  # wT [H, cols] -> [H/2, 2*cols]
        cols = wT.shape[1]
        return np.ascontiguousarray(
            wT.reshape(KT // 2, 2, 128, cols).transpose(0, 2, 1, 3)
            .reshape(H // 2, 2 * cols))

    wdecT = np.concatenate([W_dec_l, W_dec_r], 0).T        # [H, 16]
    st2v = kg[-1, :, :2].astype(c64)                       # [N, 2]
    r34 = np.concatenate([(A.astype(c64) @ st2v).T, st2v.T], 0)  # [4, N]
    shared = {
        "at": np.ascontiguousarray(A.T).astype(BF),
        "a2t": np.ascontiguousarray(A2.T).astype(np.float32).astype(BF),
        "r34": np.ascontiguousarray(r34).astype(np.float32).astype(BF),
        "wdecT": pack_pairs(wdecT).astype(F8),
        "qr": np.ascontiguousarray(qr),
        "rhs20w": np.ascontiguousarray(r20w).astype(BF),
        "st2": np.ascontiguousarray(kg[-1, :, :2]),
    }

    in_maps = []
    for c in range(NCORES):
        idx = np.concatenate([np.arange(g * H + c * HS, g * H + (c + 1) * HS)
                              for g in range(4)])
        whhT = pack_pairs(W_hh[idx, :].T).astype(F8)                  # [H/2, 2GD]
        wcT = np.ascontiguousarray(Wc[idx, :].T).astype(BF)           # [20, GD]
        wst = np.zeros((4 * H, 96), np.float32)
        wst[:, 0:2] = Wc[:, 0:2]
        wst[:, 2:4] = Wc[:, NF:NF + 2]
        wst[:, 32:40] = Wc[:, NF + 2:NF + NF]
        wst[:, 64:72] = Wc[:, 2:NF]
        wstackT = np.ascontiguousarray(wst[idx, :].T).astype(BF)
        bc = bias_all[:, idx].T                                       # [GD, NG]
        bt = np.ascontiguousarray(
            bc.reshape(MT, 128, NG).transpose(1, 0, 2).reshape(128, MT * NG))
        in_maps.append({
            "whhT": whhT, "wcT": wcT, "wstackT": wstackT, "biases": bt,
            **shared,
        })
    return in_maps


def kernel(**inputs):
    if _PROG[0] is None:
        _PROG[0] = _build_program()
    nc = _PROG[0]
    in_maps = _host_tensors(inputs)
    res = bass_utils.run_bass_kernel_spmd(
        nc, in_maps, core_ids=list(range(NCORES)))
    return np.ascontiguousarray(res.results[0]["gen"]).astype(np.float32)


# exposed for test.py profiling
def run_profiled(inputs, **kwargs):
    if _PROG[0] is None:
        _PROG[0] = _build_program()
    in_maps = _host_tensors(inputs)
    return bass_utils.run_bass_kernel_spmd(
        _PROG[0], in_maps, core_ids=list(range(NCORES)), **kwargs)

